# revision 2
# baseline (speedup 1.0000x reference)
"""Llama4-style MoE (top-1 router + 8 GLU experts + shared GLU expert) on 8
Trainium2 NeuronCores.

Strategy (fused expert-parallel): with top-1 routing every token visits
exactly one routed expert, so the shared expert is fused into it. Core e
processes expert e's tokens through a double-width GLU whose intermediate
dimension concatenates the expert and shared intermediates (2048 + 2048);
the router sigmoid scale is applied to the expert half of the intermediate
activations, which distributes over the down-projection. One SPMD pass
covers the whole batch with no separate shared-expert sweep.

Routing runs on the host as part of sharding; tokens are gathered per
expert (capacity CE=1088 >= observed max load 1078 for these shapes) and
scattered back after the pass. Matmuls run in fp16 with fp32 PSUM
accumulation. Weights are preprocessed/tiled once and cached on device.

Shapes are hardcoded for B=4, S=2048, H=I=2048, E=8.
"""

import os
import sys

os.environ.setdefault("JAX_PLATFORMS", "axon")

for _p in ("/opt/trn_rl_repo", "/root/.axon_site/_ro/trn_rl_repo"):
    if _p not in sys.path:
        sys.path.append(_p)

import numpy as np

import concourse.bass as bass  # noqa: F401  (keeps concourse init order stable)
import concourse.mybir as mybir
import concourse.tile as tile
from concourse import bacc

F16 = np.float16

P = 128
H = 2048
I2 = 4096  # fused intermediate: expert 2048 + shared 2048
E = 8
KT = H // P  # 16 k-tiles over H
MT2 = I2 // P  # 32 m-tiles over fused intermediate
MT = H // P  # 16 out-tiles over H
KT2 = I2 // P  # 32 k-tiles over fused intermediate
T_TOTAL = 8192

CE = 1088  # per-core expert-token capacity (max observed load 1078)
BLOCKS = [(0, 512), (512, 512), (1024, CE - 1024)]

_NC = None
_RUNNER = None
_W_DEV = None  # device-resident weight arrays, in runner input order
_W_KEY = None


def _build_nc(reps=1):
    dt = mybir.dt
    nc = bacc.Bacc("TRN2", target_bir_lowering=False, debug=False, num_devices=8)

    xe = nc.dram_tensor("xe", [P, KT, CE], dt.float16, kind="ExternalInput").ap()
    sce = nc.dram_tensor("sce", [P, CE], dt.float16, kind="ExternalInput").ap()
    wg = nc.dram_tensor("wg", [P, MT2, KT, P], dt.float16, kind="ExternalInput").ap()
    wu = nc.dram_tensor("wu", [P, MT2, KT, P], dt.float16, kind="ExternalInput").ap()
    wd = nc.dram_tensor("wd", [P, MT, KT2, P], dt.float16, kind="ExternalInput").ap()
    ye = nc.dram_tensor("ye", [MT, P, CE], dt.float16, kind="ExternalOutput").ap()

    with tile.TileContext(nc) as tc:
        with (
            tc.tile_pool(name="xpool", bufs=1) as xpool,
            tc.tile_pool(name="wpool", bufs=4) as wpool,
            tc.tile_pool(name="wdpool", bufs=2) as wdpool,
            tc.tile_pool(name="apool", bufs=1) as apool,
            tc.tile_pool(name="ypool", bufs=2) as ypool,
            tc.tile_pool(name="psum", bufs=2, space="PSUM") as psum,
        ):
            xe_sb = xpool.tile([P, KT, CE], dt.float16, tag="xe")
            nc.sync.dma_start(xe_sb[:], xe[:])
            sce_sb = xpool.tile([P, CE], dt.float16, tag="sce")
            nc.sync.dma_start(sce_sb[:], sce[:])
            a_sb = apool.tile([P, MT2, CE], dt.float16, tag="a")

            for _rep in range(reps):
                # ---- pass A: a = silu(Wg^T x) ----
                for m in range(MT2):
                    w_sb = wpool.tile([P, KT, P], dt.float16, tag="w")
                    nc.sync.dma_start(w_sb[:], wg[:, m])
                    ps = [
                        psum.tile([P, 512], dt.float32, tag=f"ps{ti}", name=f"ps{ti}")
                        for ti in range(len(BLOCKS))
                    ]
                    for k in range(KT):
                        lhs = w_sb[:, k, :]
                        for ti, (off, bl) in enumerate(BLOCKS):
                            nc.tensor.matmul(
                                ps[ti][:, :bl],
                                lhs,
                                xe_sb[:, k, off : off + bl],
                                start=(k == 0),
                                stop=(k == KT - 1),
                            )
                    for ti, (off, bl) in enumerate(BLOCKS):
                        nc.scalar.activation(
                            a_sb[:, m, off : off + bl],
                            ps[ti][:, :bl],
                            mybir.ActivationFunctionType.Silu,
                        )
                # ---- pass B: a *= Wu^T x; expert half also *= router scale ----
                for m in range(MT2):
                    w_sb = wpool.tile([P, KT, P], dt.float16, tag="w")
                    nc.sync.dma_start(w_sb[:], wu[:, m])
                    ps = [
                        psum.tile([P, 512], dt.float32, tag=f"ps{ti}", name=f"ps{ti}")
                        for ti in range(len(BLOCKS))
                    ]
                    for k in range(KT):
                        lhs = w_sb[:, k, :]
                        for ti, (off, bl) in enumerate(BLOCKS):
                            nc.tensor.matmul(
                                ps[ti][:, :bl],
                                lhs,
                                xe_sb[:, k, off : off + bl],
                                start=(k == 0),
                                stop=(k == KT - 1),
                            )
                    for ti, (off, bl) in enumerate(BLOCKS):
                        nc.vector.tensor_tensor(
                            a_sb[:, m, off : off + bl],
                            a_sb[:, m, off : off + bl],
                            ps[ti][:, :bl],
                            mybir.AluOpType.mult,
                        )
                        if m < MT2 // 2:  # expert half of the intermediate
                            nc.vector.tensor_tensor(
                                a_sb[:, m, off : off + bl],
                                a_sb[:, m, off : off + bl],
                                sce_sb[:, off : off + bl],
                                mybir.AluOpType.mult,
                            )
                # ---- pass C: y = Wd^T a ----
                for m in range(MT):
                    w_sb = wdpool.tile([P, KT2, P], dt.float16, tag="wd")
                    nc.sync.dma_start(w_sb[:], wd[:, m])
                    ps = [
                        psum.tile([P, 512], dt.float32, tag=f"ps{ti}", name=f"ps{ti}")
                        for ti in range(len(BLOCKS))
                    ]
                    for k in range(KT2):
                        lhs = w_sb[:, k, :]
                        for ti, (off, bl) in enumerate(BLOCKS):
                            nc.tensor.matmul(
                                ps[ti][:, :bl],
                                lhs,
                                a_sb[:, k, off : off + bl],
                                start=(k == 0),
                                stop=(k == KT2 - 1),
                            )
                    y_sb = ypool.tile([P, CE], dt.float16, tag="y")
                    for ti, (off, bl) in enumerate(BLOCKS):
                        nc.scalar.copy(y_sb[:, off : off + bl], ps[ti][:, :bl])
                    nc.sync.dma_start(ye[m], y_sb[:])
    nc.compile()
    return nc


class _Runner:
    """Compile a Bass module into a sharded jitted callable over 8 cores,
    with device-resident input caching (mirrors bass2jax.run_bass_via_pjrt
    but reusable across calls)."""

    def __init__(self, nc, n_cores=8):
        import jax
        from jax.experimental.shard_map import shard_map
        from jax.sharding import Mesh, NamedSharding, PartitionSpec

        from concourse import bass2jax

        bass2jax.install_neuronx_cc_hook()
        self.jax = jax
        self.n_cores = n_cores

        partition_name = (
            nc.partition_id_tensor.name if nc.partition_id_tensor else None
        )
        in_names, out_names, out_avals = [], [], []
        self.in_shapes = {}
        for alloc in nc.m.functions[0].allocations:
            if not isinstance(alloc, mybir.MemoryLocationSet):
                continue
            name = alloc.memorylocations[0].name
            if alloc.kind == "ExternalInput":
                if name != partition_name:
                    in_names.append(name)
                    self.in_shapes[name] = (
                        tuple(alloc.tensor_shape),
                        mybir.dt.np(alloc.dtype),
                    )
            elif alloc.kind == "ExternalOutput":
                out_names.append(name)
                out_avals.append(
                    jax.core.ShapedArray(
                        tuple(alloc.tensor_shape), mybir.dt.np(alloc.dtype)
                    )
                )
        if nc.dbg_addr is not None:
            assert not nc.dbg_callbacks
            # 8-byte PA viewed as uint32[1,2]; zeros skip the dbg store+halt
            self.in_shapes[nc.dbg_addr.name] = ((1, 2), np.uint32)

        n_params = len(in_names)
        full_in_names = tuple(
            in_names + out_names + ([partition_name] if partition_name else [])
        )

        def _body(*args):
            operands = list(args)
            if partition_name is not None:
                operands.append(bass2jax.partition_id_tensor())
            outs = bass2jax._bass_exec_p.bind(
                *operands,
                out_avals=tuple(out_avals),
                in_names=full_in_names,
                out_names=tuple(out_names),
                lowering_input_output_aliases=(),
                sim_require_finite=True,
                sim_require_nnan=True,
                nc=nc,
            )
            return tuple(outs)

        devices = jax.devices()[:n_cores]
        assert len(devices) == n_cores, (n_cores, jax.devices())
        mesh = Mesh(np.asarray(devices), ("core",))
        spec = PartitionSpec("core")
        n_all = n_params + len(out_names)
        self.fn = jax.jit(
            shard_map(
                _body,
                mesh=mesh,
                in_specs=(spec,) * n_all,
                out_specs=(spec,) * len(out_names),
                check_rep=False,
            ),
            donate_argnums=tuple(range(n_params, n_all)),
            keep_unused=True,
        )
        self.sharding = NamedSharding(mesh, spec)
        self.in_names = in_names
        self.out_names = out_names
        self.out_shapes = [(tuple(a.shape), a.dtype) for a in out_avals]

    def put(self, concat_array):
        """Upload a global (n_cores*d0, ...) array with core sharding."""
        return self.jax.device_put(np.ascontiguousarray(concat_array), self.sharding)

    def zeros_in(self, name):
        shape, dtype = self.in_shapes[name]
        return self.put(np.zeros((self.n_cores * shape[0],) + shape[1:], dtype))

    def zero_outs(self):
        return [
            self.put(np.zeros((self.n_cores * s[0],) + s[1:], d))
            for (s, d) in self.out_shapes
        ]

    def call(self, params, outs):
        return self.fn(*params, *outs)


def _get_runner():
    global _NC, _RUNNER
    if _RUNNER is None:
        _NC = _build_nc()
        _RUNNER = _Runner(_NC)
    return _RUNNER


def _tile_fused_in(w):
    """[H(K), I2(M)] -> [P, MT2, KT, P] f16 with out[p,m,k,i] = w[k*P+p, m*P+i]."""
    w = np.asarray(w, F16)
    return np.ascontiguousarray(w.reshape(KT, P, MT2, P).transpose(1, 2, 0, 3))


def _tile_fused_out(w):
    """[I2(K), H(M)] -> [P, MT, KT2, P] f16 with out[p,m,k,i] = w[k*P+p, m*P+i]."""
    w = np.asarray(w, F16)
    return np.ascontiguousarray(w.reshape(KT2, P, MT, P).transpose(1, 2, 0, 3))


def _feat_major(x):
    """[T, H] f16 -> [P, KT, T] with out[p,k,t] = x[t, k*P+p]."""
    T = x.shape[0]
    return np.ascontiguousarray(x.T.reshape(KT, P, T).transpose(1, 0, 2))


def _get_device_weights(r, w1, v1, w2, shared_gate, shared_up, shared_down):
    global _W_DEV, _W_KEY
    key = (id(w1), id(v1), id(w2), id(shared_gate), id(shared_up), id(shared_down))
    if _W_DEV is not None and _W_KEY == key:
        return _W_DEV
    w1 = np.asarray(w1)
    v1 = np.asarray(v1)
    w2 = np.asarray(w2)
    sg = np.asarray(shared_gate).T  # [I,H] -> [H,I]
    su = np.asarray(shared_up).T
    sd = np.asarray(shared_down).T  # [H,I] -> [I,H]
    per_name = {"wg": [], "wu": [], "wd": []}
    for e in range(E):
        per_name["wg"].append(_tile_fused_in(np.concatenate([w1[e], sg], axis=1)))
        per_name["wu"].append(_tile_fused_in(np.concatenate([v1[e], su], axis=1)))
        per_name["wd"].append(_tile_fused_out(np.concatenate([w2[e], sd], axis=0)))
    dev = {n: r.put(np.concatenate(per_name[n], axis=0)) for n in per_name}
    _W_DEV = dev
    _W_KEY = key
    return dev


def kernel(
    hidden_states,
    router_w,
    w1,
    v1,
    w2,
    shared_gate,
    shared_up,
    shared_down,
):
    hidden_states = np.asarray(hidden_states, dtype=np.float32)
    router_w = np.asarray(router_w, dtype=np.float32)

    B, S, _ = hidden_states.shape
    x = hidden_states.reshape(-1, H)  # [T, H]
    T = x.shape[0]

    # --- routing (host side, part of sharding) ---
    logits = x @ router_w.T  # [T, E]
    top = np.argmax(logits, axis=1)
    wt = 1.0 / (1.0 + np.exp(-logits[np.arange(T), top]))  # sigmoid(top logit)

    r = _get_runner()
    wdev = _get_device_weights(r, w1, v1, w2, shared_gate, shared_up, shared_down)
    xf = x.astype(F16)

    per_expert = [np.nonzero(top == e)[0] for e in range(E)]

    out = np.zeros((T, H), dtype=np.float32)
    first = True
    while first or any(len(ix) for ix in per_expert):
        idx_lists = [ix[:CE] for ix in per_expert]
        per_expert = [ix[CE:] for ix in per_expert]

        xe_np = np.zeros((E * P, KT, CE), dtype=F16)
        sce_np = np.zeros((E * P, CE), dtype=F16)
        for e in range(E):
            idx = idx_lists[e]
            n = len(idx)
            xe_h = np.zeros((CE, H), dtype=F16)
            if n:
                xe_h[:n] = xf[idx]
                sce_np[e * P : (e + 1) * P, :n] = wt[idx].astype(F16)
            xe_np[e * P : (e + 1) * P] = _feat_major(xe_h)

        params = []
        for name in r.in_names:
            if name == "xe":
                params.append(r.put(xe_np))
            elif name == "sce":
                params.append(r.put(sce_np))
            elif name in wdev:
                params.append(wdev[name])
            else:
                params.append(r.zeros_in(name))
        outs = r.call(params, r.zero_outs())
        ye = np.asarray(outs[r.out_names.index("ye")]).reshape(E, MT, P, CE)

        for e in range(E):
            idx = idx_lists[e]
            n = len(idx)
            if n:
                y2 = ye[e].transpose(2, 0, 1).reshape(CE, H)[:n]
                out[idx] = y2.astype(np.float32)
        first = False

    return out.reshape(B, S, H)
